# revision 4
# baseline (speedup 1.0000x reference)
"""Trainium2 Bass kernel for CayleyConvolution (gnn_message_passing).

Math restructuring
------------------
Reference computes, with hL = h*(I - adj) (REAL-valued, cast to complex):
    diag_inv = 1/(rowsum(hL) + i)
    out = x@w0 ; prev = I
    for j in 0..1:
        xj = hL@prev - i*prev
        y = xj; 3x: y = y + diag_inv*(xj - (hL@y + i*y))
        csum += y @ (x @ (wr[j] + i*wi[j]))
        prev = y
    out = relu(out + 2*Re(csum))

Both the rhs-build and Jacobi iterations are LINEAR in the N x N state, and the
state is only ever used multiplied by a narrow [N, 256] complex matrix.  With
G(v) := Jacobi3((hL - iI) @ v) applied to narrow panels, and p_j = x@(wr_j+i*wi_j):
    y_0 = G(I),  y_1 = G(y_0)  =>  csum = y_0@p_0 + y_1@p_1 = G(p_0 + G(p_1))
so the whole kernel is 8 sequential matvec steps A @ [3072, 512] (re|im panels)
plus tiny elementwise updates -- no N x N intermediates at all.

Distribution (8 NeuronCores)
----------------------------
Row-shard A: core c owns rows [c*384, (c+1)*384).  Each matvec step computes
the core's 384 output rows (needs the FULL y panel), does the Jacobi
elementwise update on its rows, then AllGathers the updated fp16 panel.
Weights/x-block matmuls (p0/p1/x@w0) are computed per-core in f32.
TensorE matvecs run in fp16 (measured end-to-end rel err ~1e-3), Jacobi state
and elementwise math stay f32.
"""

import os
import sys

import numpy as np

for _p in ("/opt/trn_rl_repo", "/root/.axon_site/_ro/trn_rl_repo"):
    if os.path.isdir(_p) and _p not in sys.path:
        sys.path.insert(0, _p)

from concourse import bacc, bass, bass_utils, mybir, tile  # noqa: E402

N = 3072
NCORES = 8
R = N // NCORES            # 384 rows per core
FIN = 512
FOUT = 256
F = 2 * FOUT               # 512 = [re | im] panel width
KT = N // 128              # 24 contraction tiles
MT = R // 128              # 3 output row tiles per core
F32 = mybir.dt.float32
F16 = mybir.dt.float16
ALU = mybir.AluOpType
NCH = ((0, 512), (512, 1024), (1024, 1280))  # p-matmul N chunks of wcat


def _emit(ctx, tc, nc, lhsA, xT, wcat, dvec, out):
    cpool = ctx.enter_context(tc.tile_pool(name="consts", bufs=1))
    work = ctx.enter_context(tc.tile_pool(name="work", bufs=4))
    psum = ctx.enter_context(tc.tile_pool(name="psum", bufs=4, space="PSUM"))
    dram = ctx.enter_context(tc.tile_pool(name="dram", bufs=1, space="DRAM"))

    # ---- constant / state tiles --------------------------------------------
    Asb = cpool.tile([128, KT, R], F16)          # A^T tiles: lhsT for matvec
    xTsb = cpool.tile([128, FIN // 128, R], F32)
    wsb = cpool.tile([128, FIN // 128, FOUT * 5], F32)
    dsb = cpool.tile([128, MT, 4], F32)          # dr | di | mdi | 0
    p0 = cpool.tile([128, MT, F], F32)
    vcur = cpool.tile([128, MT, F], F32)         # current G input panel rows
    wst = cpool.tile([128, MT, F], F32)          # Jacobi rhs w
    ya = cpool.tile([128, MT, F], F32)
    yb = cpool.tile([128, MT, F], F32)
    out0 = cpool.tile([128, MT, FOUT], F32)      # x@w0 rows
    yio = cpool.tile([128, MT, F], F16)          # fp16 panel to AllGather
    yf = cpool.tile([128, KT, F], F16)           # full gathered panel

    agin = dram.tile([R, F], F16)
    agout = dram.tile([N, F], F16)

    # ---- load inputs --------------------------------------------------------
    nc.sync.dma_start(out=Asb[:], in_=lhsA.ap().rearrange("(k p) m -> p k m", p=128))
    nc.sync.dma_start(out=xTsb[:], in_=xT.ap().rearrange("(k p) m -> p k m", p=128))
    nc.sync.dma_start(out=wsb[:], in_=wcat.ap().rearrange("(k p) m -> p k m", p=128))
    nc.sync.dma_start(out=dsb[:], in_=dvec.ap().rearrange("(m p) c -> p m c", p=128))

    # ---- p-matmuls: [x@w0 | p0re | p0im | p1re | p1im] ---------------------
    # out rows m: [384, 1280] = xT.T @ wcat, chunked along N by 512
    for m in range(MT):
        ms = slice(m * 128, (m + 1) * 128)
        for n0, n1 in NCH:
            ps = psum.tile([128, n1 - n0], F32, tag="psp")
            for k in range(FIN // 128):
                nc.tensor.matmul(
                    ps[:],
                    xTsb[:, k, ms],
                    wsb[:, k, n0:n1],
                    start=(k == 0),
                    stop=(k == FIN // 128 - 1),
                )
            if n0 == 0:
                nc.scalar.copy(out0[:, m, :], ps[:, 0:256])
                nc.vector.tensor_copy(p0[:, m, 0:256], ps[:, 256:512])
            elif n0 == 512:
                nc.vector.tensor_copy(p0[:, m, 256:512], ps[:, 0:256])
                nc.vector.tensor_copy(vcur[:, m, 0:256], ps[:, 256:512])
                nc.vector.tensor_copy(yio[:, m, 0:256], ps[:, 256:512])
            else:
                nc.vector.tensor_copy(vcur[:, m, 256:512], ps[:, 0:256])
                nc.vector.tensor_copy(yio[:, m, 256:512], ps[:, 0:256])

    # ---- 8 matvec steps -----------------------------------------------------
    # step s: AllGather yio -> yf; mm = A_c @ yf; elementwise update on rows.
    # G #1: steps 1..4 (step 1 builds w from v=p1; 2..4 Jacobi)
    # G #2: steps 5..8 (step 5 builds w from v=s=p0+G1; 6..8 Jacobi)
    agout_r = agout.rearrange("(k p) n -> p k n", p=128)
    for s in range(1, 9):
        first_of_g = s in (1, 5)
        ji = 0 if first_of_g else (s - (1 if s < 5 else 5))  # jacobi iter 1..3
        # ping-pong: state written by previous step
        if first_of_g:
            ysrc, ydst = None, ya
        else:
            ysrc = ya if ji in (1, 3) else yb
            ydst = yb if ji in (1, 3) else ya

        nc.sync.dma_start(
            out=agin.rearrange("(m p) n -> p m n", p=128), in_=yio[:]
        )
        nc.gpsimd.collective_compute(
            "AllGather",
            ALU.bypass,
            replica_groups=[list(range(NCORES))],
            ins=[agin.opt()],
            outs=[agout.opt()],
        )
        for ch in range(4):
            ks = slice(ch * 6, (ch + 1) * 6)
            nc.sync.dma_start(out=yf[:, ks, :], in_=agout_r[:, ks, :])

        for m in range(MT):
            ms = slice(m * 128, (m + 1) * 128)
            ps = psum.tile([128, F], F32, tag="psmv")
            for k in range(KT):
                nc.tensor.matmul(
                    ps[:],
                    Asb[:, k, ms],
                    yf[:, k, :],
                    start=(k == 0),
                    stop=(k == KT - 1),
                )
            if first_of_g:
                # w_re = mm_re + v_im ; w_im = mm_im - v_re ; y = w
                nc.vector.tensor_tensor(
                    wst[:, m, 0:256], ps[:, 0:256], vcur[:, m, 256:512], ALU.add
                )
                nc.vector.tensor_tensor(
                    wst[:, m, 256:512], ps[:, 256:512], vcur[:, m, 0:256], ALU.subtract
                )
                nc.scalar.copy(ydst[:, m, :], wst[:, m, :])
                nc.vector.tensor_copy(yio[:, m, :], wst[:, m, :])
            else:
                # t = w - mm (+ y_im | - y_re) ; y' = y + dinv*t (complex)
                q = work.tile([128, F], F32, tag="q")
                t = work.tile([128, F], F32, tag="t")
                u = work.tile([128, F], F32, tag="u")
                nc.vector.scalar_tensor_tensor(
                    q[:], ps[:], -1.0, wst[:, m, :], ALU.mult, ALU.add
                )
                nc.vector.tensor_tensor(
                    t[:, 0:256], q[:, 0:256], ysrc[:, m, 256:512], ALU.add
                )
                nc.vector.tensor_tensor(
                    t[:, 256:512], q[:, 256:512], ysrc[:, m, 0:256], ALU.subtract
                )
                # u = y + dr*t
                nc.vector.scalar_tensor_tensor(
                    u[:], t[:], dsb[:, m, 0:1], ysrc[:, m, :], ALU.mult, ALU.add
                )
                # y_re' = u_re + (-di)*t_im ; y_im' = u_im + di*t_re
                nc.vector.scalar_tensor_tensor(
                    ydst[:, m, 0:256], t[:, 256:512], dsb[:, m, 2:3], u[:, 0:256],
                    ALU.mult, ALU.add,
                )
                nc.vector.scalar_tensor_tensor(
                    ydst[:, m, 256:512], t[:, 0:256], dsb[:, m, 1:2], u[:, 256:512],
                    ALU.mult, ALU.add,
                )
                if s == 4:
                    # s-panel for G#2: v = p0 + G1(p1)
                    nc.vector.tensor_tensor(
                        vcur[:, m, :], p0[:, m, :], ydst[:, m, :], ALU.add
                    )
                    nc.vector.tensor_copy(yio[:, m, :], vcur[:, m, :])
                elif s == 8:
                    # out rows = relu(out0 + 2*y_re)
                    o = work.tile([128, FOUT], F32, tag="o")
                    o2 = work.tile([128, FOUT], F32, tag="o2")
                    nc.vector.scalar_tensor_tensor(
                        o[:], ydst[:, m, 0:256], 2.0, out0[:, m, :], ALU.mult, ALU.add
                    )
                    nc.vector.tensor_relu(o2[:], o[:])
                    nc.sync.dma_start(
                        out=out.ap().rearrange("(m p) n -> p m n", p=128)[:, m, :],
                        in_=o2[:],
                    )
                else:
                    nc.vector.tensor_copy(yio[:, m, :], ydst[:, m, :])

_NC_CACHE = {}


def _build():
    if "nc" in _NC_CACHE:
        return _NC_CACHE["nc"]
    nc = bacc.Bacc("TRN2", target_bir_lowering=False, debug=False, num_devices=NCORES)
    lhsA = nc.dram_tensor("lhsA", [N, R], F16, kind="ExternalInput")
    xT = nc.dram_tensor("xT", [FIN, R], F32, kind="ExternalInput")
    wcat = nc.dram_tensor("wcat", [FIN, FOUT * 5], F32, kind="ExternalInput")
    dvec = nc.dram_tensor("dvec", [R, 4], F32, kind="ExternalInput")
    out = nc.dram_tensor("out", [R, FOUT], F32, kind="ExternalOutput")
    from contextlib import ExitStack

    with tile.TileContext(nc) as tc, ExitStack() as ctx:
        _emit(ctx, tc, nc, lhsA, xT, wcat, dvec, out)
    nc.compile()
    _NC_CACHE["nc"] = nc
    return nc


def _prepare_in_maps(x, adj, h, w0, wr, wi):
    x = np.asarray(x, dtype=np.float32)
    adj = np.asarray(adj, dtype=np.float32)
    h = float(np.asarray(h))
    w0 = np.asarray(w0, dtype=np.float32)
    wr = np.asarray(wr, dtype=np.float32)
    wi = np.asarray(wi, dtype=np.float32)

    # A^T = h*(I - adj)^T, fp16, sliced into per-core column blocks
    AT = (-h) * adj.T
    AT[np.arange(N), np.arange(N)] += h
    AT16 = AT.astype(np.float16)

    d = h * (1.0 - adj.sum(axis=1))
    den = d * d + 1.0
    dvec = np.zeros((N, 4), dtype=np.float32)
    dvec[:, 0] = d / den          # Re(1/(d+i))
    dvec[:, 1] = -1.0 / den       # Im(1/(d+i))
    dvec[:, 2] = 1.0 / den        # -Im

    wcat = np.concatenate([w0, wr[0], wi[0], wr[1], wi[1]], axis=1)
    wcat = np.ascontiguousarray(wcat, dtype=np.float32)

    in_maps = []
    for c in range(NCORES):
        rs = slice(c * R, (c + 1) * R)
        in_maps.append(
            {
                "lhsA": np.ascontiguousarray(AT16[:, rs]),
                "xT": np.ascontiguousarray(x[rs].T),
                "wcat": wcat,
                "dvec": np.ascontiguousarray(dvec[rs]),
            }
        )
    return in_maps


def kernel(x, adj, h, w0, wr, wi):
    nc = _build()
    in_maps = _prepare_in_maps(x, adj, h, w0, wr, wi)
    res = bass_utils.run_bass_kernel_spmd(nc, in_maps, core_ids=list(range(NCORES)))
    out = np.concatenate([res.results[c]["out"] for c in range(NCORES)], axis=0)
    return np.ascontiguousarray(out, dtype=np.float32)


# revision 32
# speedup vs baseline: 708.4891x; 708.4891x over previous
"""Trainium2 Bass kernel for CayleyConvolution (gnn_message_passing).

Math restructuring
------------------
Reference computes, with hL = h*(I - adj) (REAL-valued, cast to complex):
    diag_inv = 1/(rowsum(hL) + i)
    out = x@w0 ; prev = I
    for j in 0..1:
        xj = hL@prev - i*prev
        y = xj; 3x: y = y + diag_inv*(xj - (hL@y + i*y))
        csum += y @ (x @ (wr[j] + i*wi[j]))
        prev = y
    out = relu(out + 2*Re(csum))

The rhs-build and Jacobi iterations are LINEAR in the N x N state, and the
state is only ever used multiplied by a narrow [N, 256] complex matrix.  With
G(v) := Jacobi3((hL - iI) @ v) applied to narrow panels, and p_j = x@(wr_j+i*wi_j):
    y_0 = G(I),  y_1 = G(y_0)  =>  csum = y_0@p_0 + y_1@p_1 = G(p_0 + G(p_1))
so the whole kernel is 8 sequential matvec steps A @ [3072, 512] (re|im panels)
plus tiny elementwise updates -- no N x N intermediates at all.

Distribution (8 NeuronCores)
----------------------------
Row-shard A: core c owns rows [c*384, (c+1)*384).  Each matvec step computes
the core's 384 output rows (needs the FULL panel), does the Jacobi elementwise
update on its rows, then AllGathers the updated fp16 panel.  The complex
columns are split into `CHAINS` independent chains, interleaved so one chain's
AllGather/DMA hides behind the other's TensorE matvecs.  TensorE runs fp16
(measured end-to-end rel err ~1e-3); Jacobi state and elementwise math stay f32.
"""

import os
import sys
from contextlib import ExitStack

import numpy as np

for _p in ("/opt/trn_rl_repo", "/root/.axon_site/_ro/trn_rl_repo"):
    if os.path.isdir(_p) and _p not in sys.path:
        sys.path.insert(0, _p)

from concourse import bacc, bass, bass_utils, mybir, tile  # noqa: E402

N = 3072
NCORES = 8
R = N // NCORES            # 384 rows per core
FIN = 512
FOUT = 256
KT = N // 128              # 24 contraction tiles
MT = R // 128              # 3 output row tiles per core
F32 = mybir.dt.float32
F16 = mybir.dt.float16
ALU = mybir.AluOpType
NCH = ((0, 512), (512, 1024), (1024, 1280))  # p-matmul N chunks of wcat
CHAINS = 2


def _emit(ctx, tc, nc, lhsA, xT, wcat, dvec, out, nrep=1, no_collective=False,
          chains=CHAINS):
    CH = chains
    WC = FOUT // CH          # complex columns per chain
    PW = 2 * WC              # f32 panel width per chain: [re | im]

    cpool = ctx.enter_context(tc.tile_pool(name="consts", bufs=1))
    work = ctx.enter_context(tc.tile_pool(name="work", bufs=4))
    psum = ctx.enter_context(tc.tile_pool(name="psum", bufs=4, space="PSUM"))
    dram = ctx.enter_context(tc.tile_pool(name="dram", bufs=1, space="DRAM"))

    # ---- constant / state tiles --------------------------------------------
    Asb = cpool.tile([128, KT, R], F16)          # A^T tiles: lhsT for matvec
    xTsb = cpool.tile([128, FIN // 128, R], F32)
    wsb = cpool.tile([128, FIN // 128, FOUT * 5], F32)
    dsb = cpool.tile([128, MT, 4], F32)          # dr | di | mdi | 0
    out0 = cpool.tile([128, MT, FOUT], F32)      # x@w0 rows

    # per-chain tiles: chains must NOT share tiles, or coarse-grained tile
    # dependency tracking serializes one chain's AllGather against the other
    # chain's compute.
    def per_chain(name, shape, dt):
        return [
            cpool.tile(shape, dt, name=f"{name}{h}") for h in range(CH)
        ]

    p0 = per_chain("p0", [128, MT, PW], F32)     # [re | im] panels
    vcur = per_chain("vcur", [128, MT, PW], F32)  # current G input panel rows
    wst = per_chain("wst", [128, MT, PW], F32)   # Jacobi rhs w
    ya = per_chain("ya", [128, MT, PW], F32)
    yb = per_chain("yb", [128, MT, PW], F32)
    yio = per_chain("yio", [128, MT, PW], F16)   # fp16 panels to AllGather
    yf = per_chain("yf", [128, KT, PW], F16)     # full gathered panels

    agin = [dram.tile([R, PW], F16, name=f"agin{h}") for h in range(CH)]
    agout = [dram.tile([N, PW], F16, name=f"agout{h}") for h in range(CH)]

    # ---- load inputs --------------------------------------------------------
    nc.sync.dma_start(out=Asb[:], in_=lhsA.ap().rearrange("(k p) m -> p k m", p=128))
    nc.sync.dma_start(out=xTsb[:], in_=xT.ap().rearrange("(k p) m -> p k m", p=128))
    nc.sync.dma_start(out=wsb[:], in_=wcat.ap().rearrange("(k p) m -> p k m", p=128))
    nc.sync.dma_start(out=dsb[:], in_=dvec.ap().rearrange("(m p) c -> p m c", p=128))

    # ---- p-matmuls: [x@w0 | p0re | p0im | p1re | p1im] ---------------------
    # wcat chunk (512,1024) = [p0im | p1re], (1024,1280) = [p1im] feed the
    # first AllGather (p1 -> yio); emit those first.  Chunk (0,512) =
    # [x@w0 | p0re] is only needed at steps 4/8 and runs under the first AG.
    def _pmm(m, n0, n1):
        ms = slice(m * 128, (m + 1) * 128)
        ps = psum.tile([128, n1 - n0], F32, tag="psp", name=f"psp{m}_{n0}")
        for k in range(FIN // 128):
            nc.tensor.matmul(
                ps[:],
                xTsb[:, k, ms],
                wsb[:, k, n0:n1],
                start=(k == 0),
                stop=(k == FIN // 128 - 1),
            )

        def scat(dst, dst_ri, src_base, cast_yio=False):
            # dst_ri: 0 for re half, 1 for im half of each chain panel
            for h in range(CH):
                src = ps[:, src_base + h * WC: src_base + (h + 1) * WC]
                nc.vector.tensor_copy(
                    dst[h][:, m, dst_ri * WC:(dst_ri + 1) * WC], src
                )
                if cast_yio:
                    nc.vector.tensor_copy(
                        yio[h][:, m, dst_ri * WC:(dst_ri + 1) * WC], src
                    )

        if n0 == 0:
            nc.scalar.copy(out0[:, m, :], ps[:, 0:256])
            scat(p0, 0, 256)                      # p0 re
        elif n0 == 512:
            scat(p0, 1, 0)                        # p0 im
            scat(vcur, 0, 256, cast_yio=True)     # p1 re
        else:
            scat(vcur, 1, 0, cast_yio=True)       # p1 im

    for m in range(MT):
        _pmm(m, 512, 1024)
        _pmm(m, 1024, 1280)
    for m in range(MT):
        _pmm(m, 0, 512)

    # ---- 8 matvec steps x nrep, CH interleaved chains ----------------------
    # G #1: steps 1..4 (step 1 builds w from v=p1; 2..4 Jacobi)
    # G #2: steps 5..8 (step 5 builds w from v=s=p0+G1; 6..8 Jacobi)
    agout_r = [a.rearrange("(k p) n -> p k n", p=128) for a in agout]
    nyf_chunks = 2
    kchunk = KT // nyf_chunks
    for s in _step_seq(nrep):
        first_of_g = s in (1, 5)
        ji = 0 if first_of_g else (s - (1 if s < 5 else 5))
        if first_of_g:
            ysrc, ydst = None, ya
        else:
            ysrc = ya if ji in (1, 3) else yb
            ydst = yb if ji in (1, 3) else ya

        for h in range(CH):
            # -- AllGather chain h's updated panel.
            nc.sync.dma_start(
                out=agin[h].rearrange("(m p) n -> p m n", p=128),
                in_=yio[h][:],
            )
            if no_collective:
                nc.sync.dma_start(out=agout[h][0:R, :], in_=agin[h][:])
            else:
                nc.gpsimd.collective_compute(
                    "AllGather",
                    ALU.bypass,
                    replica_groups=[list(range(NCORES))],
                    ins=[agin[h].opt()],
                    outs=[agout[h].opt()],
                )
            for c0 in range(0, KT, kchunk):
                nc.sync.dma_start(
                    out=yf[h][:, c0:c0 + kchunk, :],
                    in_=agout_r[h][:, c0:c0 + kchunk, :],
                )

            # -- matvec + elementwise update for chain h
            for m in range(MT):
                ms = slice(m * 128, (m + 1) * 128)
                ps = psum.tile([128, PW], F32, tag="psmv")
                for k in range(KT):
                    nc.tensor.matmul(
                        ps[:],
                        Asb[:, k, ms],
                        yf[h][:, k, :],
                        start=(k == 0),
                        stop=(k == KT - 1),
                    )
                re = slice(0, WC)
                im = slice(WC, PW)
                if first_of_g:
                    # w_re = mm_re + v_im ; w_im = mm_im - v_re ; y = w
                    nc.vector.tensor_tensor(
                        wst[h][:, m, re], ps[:, re], vcur[h][:, m, im], ALU.add
                    )
                    nc.vector.tensor_tensor(
                        wst[h][:, m, im], ps[:, im], vcur[h][:, m, re], ALU.subtract
                    )
                    nc.scalar.copy(ydst[h][:, m, :], wst[h][:, m, :])
                    nc.vector.tensor_copy(yio[h][:, m, :], wst[h][:, m, :])
                else:
                    # t = w - mm (+ y_im | - y_re) ; y' = y + dinv*t (complex)
                    q = work.tile([128, PW], F32, tag="q")
                    t = work.tile([128, PW], F32, tag="t")
                    u = work.tile([128, PW], F32, tag="u")
                    nc.vector.scalar_tensor_tensor(
                        q[:], ps[:], -1.0, wst[h][:, m, :], ALU.mult, ALU.add
                    )
                    nc.vector.tensor_tensor(
                        t[:, re], q[:, re], ysrc[h][:, m, im], ALU.add
                    )
                    nc.vector.tensor_tensor(
                        t[:, im], q[:, im], ysrc[h][:, m, re], ALU.subtract
                    )
                    # u = y + dr*t
                    nc.vector.scalar_tensor_tensor(
                        u[:], t[:], dsb[:, m, 0:1], ysrc[h][:, m, :],
                        ALU.mult, ALU.add,
                    )
                    # y_re' = u_re + (-di)*t_im ; y_im' = u_im + di*t_re
                    nc.vector.scalar_tensor_tensor(
                        ydst[h][:, m, re], t[:, im], dsb[:, m, 2:3], u[:, re],
                        ALU.mult, ALU.add,
                    )
                    nc.vector.scalar_tensor_tensor(
                        ydst[h][:, m, im], t[:, re], dsb[:, m, 1:2], u[:, im],
                        ALU.mult, ALU.add,
                    )
                    if s == 4:
                        # s-panel for G#2: v = p0 + G1(p1)
                        nc.vector.tensor_tensor(
                            vcur[h][:, m, :], p0[h][:, m, :], ydst[h][:, m, :],
                            ALU.add,
                        )
                        nc.vector.tensor_copy(yio[h][:, m, :], vcur[h][:, m, :])
                    elif s == 8:
                        # out rows = relu(out0 + 2*y_re)
                        o = work.tile([128, WC], F32, tag="o")
                        o2 = work.tile([128, WC], F32, tag="o2")
                        ocols = slice(h * WC, (h + 1) * WC)
                        nc.vector.scalar_tensor_tensor(
                            o[:], ydst[h][:, m, re], 2.0, out0[:, m, ocols],
                            ALU.mult, ALU.add,
                        )
                        nc.vector.tensor_relu(o2[:], o[:])
                        nc.scalar.dma_start(
                            out=out.ap().rearrange("(m p) n -> p m n", p=128)[
                                :, m, ocols
                            ],
                            in_=o2[:],
                        )
                    else:
                        nc.vector.tensor_copy(yio[h][:, m, :], ydst[h][:, m, :])


def _step_seq(nrep):
    for _ in range(nrep):
        yield from range(1, 9)


# ---------------------------------------------------------------------------
# 2-row x 4-col layout: core id = 2*g + r owns row-half r (1536 rows of A)
# and column-group g (64 complex cols).  Per step the only communication is a
# 2-rank AllGather with the row partner (pairs [2g, 2g+1]); the local-K half
# of the matvec overlaps it.  lhsA is shipped with K reordered [own-half
# rows; partner-half rows] so both matmul phases use static k-tile ranges;
# the single runtime-variant quantity (where the partner block lands in the
# gathered output) is a dynamic DMA offset derived from partition_id.
# Exchange buffers are partition-major ([128, m*n] rows of 3KB) so the AG
# stacks rank blocks on axis 0 and all DMAs are fully coalesced.
# ---------------------------------------------------------------------------
RG = 2
CG = 4
R2 = N // RG               # 1536 rows per core
MT2 = R2 // 128            # 12 m-tiles
KL = KT // RG              # 12 local k-tiles
WC2 = FOUT // CG           # 64 complex cols per core
PW2 = 2 * WC2              # 128 f32 panel cols
BANKS = 6                  # matvec psum banks; 2 m-tiles packed per bank
MPB = MT2 // BANKS


def _emit24(ctx, tc, nc, lhsA, xT, wcat, dvec, out, nrep=1, no_collective=False):
    cpool = ctx.enter_context(tc.tile_pool(name="consts", bufs=1))
    work = ctx.enter_context(tc.tile_pool(name="work", bufs=6))
    psum = ctx.enter_context(tc.tile_pool(name="psum", bufs=7, space="PSUM"))
    psp = ctx.enter_context(tc.tile_pool(name="psp", bufs=1, space="PSUM"))
    dram = ctx.enter_context(tc.tile_pool(name="dram", bufs=1, space="DRAM"))

    Asb = cpool.tile([128, KT, R2], F16)
    xTsb = cpool.tile([128, FIN // 128, R2], F32)
    wsb = cpool.tile([128, FIN // 128, WC2 * 5], F32)
    dsb = cpool.tile([128, MT2, 4], F32)
    out0 = cpool.tile([128, MT2, WC2], F32)
    p0 = cpool.tile([128, MT2, PW2], F32)
    vcur = cpool.tile([128, MT2, PW2], F32)
    wst = cpool.tile([128, MT2, PW2], F32)
    ya = cpool.tile([128, MT2, PW2], F32)
    yb = cpool.tile([128, MT2, PW2], F32)
    yio = cpool.tile([128, MT2, PW2], F16)   # own half (also local-phase rhs)
    yfr = cpool.tile([128, KL, PW2], F16)    # partner half

    agin = dram.tile([128, MT2 * PW2], F16)
    agout = dram.tile([2 * 128, MT2 * PW2], F16)

    # partner block row base in agout: 128 * (1 - (core_id & 1))
    pid = nc.gpsimd.partition_id()
    prow = nc.gpsimd.snap((1 - (pid & 1)) * 128)

    # ---- load inputs (small tensors first; lhsA chunked, early k first) ----
    nc.sync.dma_start(out=xTsb[:], in_=xT.ap().rearrange("(k p) m -> p k m", p=128))
    nc.sync.dma_start(out=wsb[:], in_=wcat.ap().rearrange("(k p) m -> p k m", p=128))
    nc.sync.dma_start(out=dsb[:], in_=dvec.ap().rearrange("(m p) c -> p m c", p=128))
    lhsA_r = lhsA.ap().rearrange("(k p) m -> p k m", p=128)
    for c0 in range(0, KT, 6):
        nc.sync.dma_start(out=Asb[:, c0:c0 + 6, :], in_=lhsA_r[:, c0:c0 + 6, :])

    # ---- p-matmuls: out rows x [x@w0 | p0re | p0im | p1re | p1im] ----------
    re = slice(0, WC2)
    im = slice(WC2, PW2)
    for m in range(MT2):
        ms = slice(m * 128, (m + 1) * 128)
        ps = psp.tile([128, WC2 * 5], F32, tag="psp", name=f"psp{m}")
        for k in range(FIN // 128):
            nc.tensor.matmul(
                ps[:], xTsb[:, k, ms], wsb[:, k, :],
                start=(k == 0), stop=(k == FIN // 128 - 1),
            )
        nc.scalar.copy(out0[:, m, :], ps[:, 0:WC2])
        nc.vector.tensor_copy(p0[:, m, re], ps[:, WC2:2 * WC2])
        nc.vector.tensor_copy(p0[:, m, im], ps[:, 2 * WC2:3 * WC2])
        nc.vector.tensor_copy(vcur[:, m, re], ps[:, 3 * WC2:4 * WC2])
        nc.vector.tensor_copy(vcur[:, m, im], ps[:, 4 * WC2:5 * WC2])
        nc.vector.tensor_copy(yio[:, m, :], ps[:, 3 * WC2:5 * WC2])

    # ---- 8 matvec steps x nrep ---------------------------------------------
    for s in _step_seq(nrep):
        first_of_g = s in (1, 5)
        ji = 0 if first_of_g else (s - (1 if s < 5 else 5))
        if first_of_g:
            ysrc, ydst = None, ya
        else:
            ysrc = ya if ji in (1, 3) else yb
            ydst = yb if ji in (1, 3) else ya

        # exchange own half (AG with row partner) -- overlaps local phase
        nc.sync.dma_start(out=agin[:], in_=yio[:])
        if no_collective:
            nc.sync.dma_start(out=agout[0:128, :], in_=agin[:])
        else:
            nc.gpsimd.collective_compute(
                "AllGather",
                ALU.bypass,
                replica_groups=[[2 * g, 2 * g + 1] for g in range(CG)],
                ins=[agin.opt()],
                outs=[agout.opt()],
            )

        pst = [
            psum.tile([128, 512], F32, tag="mv", name=f"mv{s}_{b}")
            for b in range(BANKS)
        ]

        # local phase: k-tiles 0..KL-1 (own rows; rhs is resident yio)
        for b in range(BANKS):
            for mi in range(MPB):
                m = b * MPB + mi
                ms = slice(m * 128, (m + 1) * 128)
                on = slice(mi * 128, (mi + 1) * 128)
                for kk in range(KL):
                    # start=True zeroes the whole 2KB psum region, so only
                    # the bank's very first matmul starts; per-element
                    # has_written makes the other column-groups accumulate
                    # from zero correctly.
                    nc.tensor.matmul(
                        pst[b][:, on], Asb[:, kk, ms], yio[:, kk, :],
                        start=(mi == 0 and kk == 0), stop=False,
                    )

        # partner half arrives; dynamic offset selects its block in agout
        nc.gpsimd.dma_start(
            out=yfr[:].rearrange("p k n -> p (k n)"),
            in_=agout[bass.ds(prow, 128), :],
        )

        # remote phase: k-tiles KL..KT-1 (partner rows)
        for b in range(BANKS):
            for mi in range(MPB):
                m = b * MPB + mi
                ms = slice(m * 128, (m + 1) * 128)
                on = slice(mi * 128, (mi + 1) * 128)
                for kk in range(KL):
                    nc.tensor.matmul(
                        pst[b][:, on], Asb[:, KL + kk, ms], yfr[:, kk, :],
                        start=False, stop=(mi == MPB - 1 and kk == KL - 1),
                    )

            # elementwise updates for this bank's m-tiles
            for mi in range(MPB):
                m = b * MPB + mi
                on0 = mi * 128
                psre = pst[b][:, on0:on0 + WC2]
                psim = pst[b][:, on0 + WC2:on0 + PW2]
                if first_of_g:
                    nc.vector.tensor_tensor(
                        wst[:, m, re], psre, vcur[:, m, im], ALU.add
                    )
                    nc.vector.tensor_tensor(
                        wst[:, m, im], psim, vcur[:, m, re], ALU.subtract
                    )
                    nc.scalar.copy(ydst[:, m, :], wst[:, m, :])
                    nc.vector.tensor_copy(yio[:, m, :], wst[:, m, :])
                else:
                    q = work.tile([128, PW2], F32, tag="q", name=f"q{s}_{m}")
                    t = work.tile([128, PW2], F32, tag="t", name=f"t{s}_{m}")
                    u = work.tile([128, PW2], F32, tag="u", name=f"u{s}_{m}")
                    nc.vector.scalar_tensor_tensor(
                        q[:], pst[b][:, on0:on0 + PW2], -1.0, wst[:, m, :],
                        ALU.mult, ALU.add,
                    )
                    nc.vector.tensor_tensor(
                        t[:, re], q[:, re], ysrc[:, m, im], ALU.add
                    )
                    nc.vector.tensor_tensor(
                        t[:, im], q[:, im], ysrc[:, m, re], ALU.subtract
                    )
                    nc.vector.scalar_tensor_tensor(
                        u[:], t[:], dsb[:, m, 0:1], ysrc[:, m, :],
                        ALU.mult, ALU.add,
                    )
                    nc.vector.scalar_tensor_tensor(
                        ydst[:, m, re], t[:, im], dsb[:, m, 2:3], u[:, re],
                        ALU.mult, ALU.add,
                    )
                    nc.vector.scalar_tensor_tensor(
                        ydst[:, m, im], t[:, re], dsb[:, m, 1:2], u[:, im],
                        ALU.mult, ALU.add,
                    )
                    if s == 4:
                        nc.vector.tensor_tensor(
                            vcur[:, m, :], p0[:, m, :], ydst[:, m, :], ALU.add
                        )
                        nc.vector.tensor_copy(yio[:, m, :], vcur[:, m, :])
                    elif s == 8:
                        o = work.tile([128, WC2], F32, tag="o", name=f"o{s}_{m}")
                        o2 = work.tile([128, WC2], F32, tag="o2",
                                       name=f"o2{s}_{m}")
                        nc.vector.scalar_tensor_tensor(
                            o[:], ydst[:, m, re], 2.0, out0[:, m, :],
                            ALU.mult, ALU.add,
                        )
                        nc.vector.tensor_relu(o2[:], o[:])
                        nc.scalar.dma_start(
                            out=out.ap().rearrange("(m p) n -> p m n", p=128)[
                                :, m, :
                            ],
                            in_=o2[:],
                        )
                    else:
                        nc.vector.tensor_copy(yio[:, m, :], ydst[:, m, :])


_NC_CACHE = {}


def _build(nrep=1, no_collective=False, chains=CHAINS, layout="2x4"):
    key = (nrep, no_collective, chains, layout)
    if key in _NC_CACHE:
        return _NC_CACHE[key]
    nc = bacc.Bacc("TRN2", target_bir_lowering=False, debug=False, num_devices=NCORES)
    if layout == "2x4":
        lhsA = nc.dram_tensor("lhsA", [N, R2], F16, kind="ExternalInput")
        xT = nc.dram_tensor("xT", [FIN, R2], F32, kind="ExternalInput")
        wcat = nc.dram_tensor("wcat", [FIN, WC2 * 5], F32, kind="ExternalInput")
        dvec = nc.dram_tensor("dvec", [R2, 4], F32, kind="ExternalInput")
        out = nc.dram_tensor("out", [R2, WC2], F32, kind="ExternalOutput")
        with tile.TileContext(nc) as tc, ExitStack() as ctx:
            _emit24(ctx, tc, nc, lhsA, xT, wcat, dvec, out,
                    nrep=nrep, no_collective=no_collective)
    else:
        lhsA = nc.dram_tensor("lhsA", [N, R], F16, kind="ExternalInput")
        xT = nc.dram_tensor("xT", [FIN, R], F32, kind="ExternalInput")
        wcat = nc.dram_tensor("wcat", [FIN, FOUT * 5], F32, kind="ExternalInput")
        dvec = nc.dram_tensor("dvec", [R, 4], F32, kind="ExternalInput")
        out = nc.dram_tensor("out", [R, FOUT], F32, kind="ExternalOutput")
        with tile.TileContext(nc) as tc, ExitStack() as ctx:
            _emit(ctx, tc, nc, lhsA, xT, wcat, dvec, out,
                  nrep=nrep, no_collective=no_collective, chains=chains)
    nc.compile()
    _NC_CACHE[key] = nc
    return nc


def _prep_common(x, adj, h, w0, wr, wi):
    x = np.asarray(x, dtype=np.float32)
    adj = np.asarray(adj, dtype=np.float32)
    h = float(np.asarray(h))
    w0 = np.asarray(w0, dtype=np.float32)
    wr = np.asarray(wr, dtype=np.float32)
    wi = np.asarray(wi, dtype=np.float32)
    AT = (-h) * adj.T
    AT[np.arange(N), np.arange(N)] += h
    AT16 = AT.astype(np.float16)
    d = h * (1.0 - adj.sum(axis=1))
    den = d * d + 1.0
    dvec = np.zeros((N, 4), dtype=np.float32)
    dvec[:, 0] = d / den
    dvec[:, 1] = -1.0 / den
    dvec[:, 2] = 1.0 / den
    return x, w0, wr, wi, AT16, dvec


def _prepare_in_maps24(x, adj, h, w0, wr, wi):
    x, w0, wr, wi, AT16, dvec = _prep_common(x, adj, h, w0, wr, wi)
    in_maps = []
    for c in range(NCORES):
        g, r = c // 2, c % 2
        rows = slice(r * R2, (r + 1) * R2)
        prows = slice((1 - r) * R2, (2 - r) * R2)
        gc = slice(g * WC2, (g + 1) * WC2)
        # K reordered: own-half rows first, then partner-half rows
        lhsA = np.concatenate(
            [AT16[rows, :][:, rows], AT16[prows, :][:, rows]], axis=0
        )
        wcat = np.concatenate(
            [w0[:, gc], wr[0][:, gc], wi[0][:, gc], wr[1][:, gc],
             wi[1][:, gc]], axis=1
        )
        in_maps.append(
            {
                "lhsA": np.ascontiguousarray(lhsA),
                "xT": np.ascontiguousarray(x[rows].T),
                "wcat": np.ascontiguousarray(wcat, dtype=np.float32),
                "dvec": np.ascontiguousarray(dvec[rows]),
            }
        )
    return in_maps


def _assemble24(results):
    out = np.empty((N, FOUT), dtype=np.float32)
    for c in range(NCORES):
        g, r = c // 2, c % 2
        out[r * R2:(r + 1) * R2, g * WC2:(g + 1) * WC2] = results[c]["out"]
    return out


def _prepare_in_maps(x, adj, h, w0, wr, wi):
    x = np.asarray(x, dtype=np.float32)
    adj = np.asarray(adj, dtype=np.float32)
    h = float(np.asarray(h))
    w0 = np.asarray(w0, dtype=np.float32)
    wr = np.asarray(wr, dtype=np.float32)
    wi = np.asarray(wi, dtype=np.float32)

    # A^T = h*(I - adj)^T, fp16, sliced into per-core column blocks
    AT = (-h) * adj.T
    AT[np.arange(N), np.arange(N)] += h
    AT16 = AT.astype(np.float16)

    d = h * (1.0 - adj.sum(axis=1))
    den = d * d + 1.0
    dvec = np.zeros((N, 4), dtype=np.float32)
    dvec[:, 0] = d / den          # Re(1/(d+i))
    dvec[:, 1] = -1.0 / den       # Im(1/(d+i))
    dvec[:, 2] = 1.0 / den        # -Im

    wcat = np.concatenate([w0, wr[0], wi[0], wr[1], wi[1]], axis=1)
    wcat = np.ascontiguousarray(wcat, dtype=np.float32)

    in_maps = []
    for c in range(NCORES):
        rs = slice(c * R, (c + 1) * R)
        in_maps.append(
            {
                "lhsA": np.ascontiguousarray(AT16[:, rs]),
                "xT": np.ascontiguousarray(x[rs].T),
                "wcat": wcat,
                "dvec": np.ascontiguousarray(dvec[rs]),
            }
        )
    return in_maps


def kernel(x, adj, h, w0, wr, wi):
    nc = _build()
    in_maps = _prepare_in_maps24(x, adj, h, w0, wr, wi)
    res = bass_utils.run_bass_kernel_spmd(nc, in_maps, core_ids=list(range(NCORES)))
    return np.ascontiguousarray(_assemble24(res.results), dtype=np.float32)
